# revision 4
# baseline (speedup 1.0000x reference)
"""AttentionNetPooling on 8 Trainium2 NeuronCores — v2 (slot redesign).

Math: scores = MLP(z); w = exp(scores + b2) (softmax denominator resolved on
host via shift-invariance); out[g] = sum_{i in g} w_i * z[i, :256] / (S * count[g]).

Design (evolved via ntff trace analysis):
 - No graph windows: each core takes a contiguous node-balanced range
   (25000 nodes).  Graphs split across cores/groups yield partial sums the
   host adds during the slot scatter.
 - Narrow one-hot: each 2048-node group uses a 64-graph local window
   (sorted batch_index => a group spans ~22 graphs).  The 0/1 one-hot is
   HOST-precomputed and shipped as fp8e4 (exact for 0/1, +1.6 MB DMA);
   on-device only the w-scale multiply remains (GpSimd tensor_tensor with
   a step-0 broadcast AP).
 - Scores via replicated-W2 stationary: one [128,512] matmul per
   superchunk; DVE 32x32 block-transpose of the raw f32 scores + 4 strided
   [32, 4S] copies per group assemble node-major score columns; ONE exp
   per group on [128, 4S].
 - Softmax denominator via exp's accum_out (free): [128, ngrp] partial
   sums DMAed out once; host subtracts the analytic pad contribution
   exp(W2 relu(b1) + b2) * npads.
 - Depth-2 software pipeline: iteration k runs scores/one-hot for group
   k-1, MLP for group k, segment matmuls for group k-2 — the one-hot
   chain gets a full group-period of slack, so the 16 segment LDW+MM
   pairs hit the PE queue with their inputs already resolved (LDWEIGHTS
   pull-ahead can hide the loads).
 - Per-group [64, 256] f32 slot output, host scatters onto global graphs.
 - HAM: warm-up matmuls + a few [128,1]-stationary fillers per group.
"""
import numpy as np
import ml_dtypes

import concourse.bass as bass
import concourse.bacc as bacc
import concourse.tile as tile
import concourse.mybir as mybir
from concourse.bass_utils import run_bass_kernel_spmd

F32 = mybir.dt.float32
BF16 = mybir.dt.bfloat16
FP8 = mybir.dt.float8e4
AF = mybir.ActivationFunctionType
ALU = mybir.AluOpType

NCORES = 8
P = 128
IN_DIM = 320
POOL = 256
HID = 128
SCG = 4            # superchunks per group (2048 nodes)
GW = 64            # graph-window (slot) width per group
SCW = 2048         # zc cols per superchunk: 1024 feat-major + 1024 node-major
NWARM = 20         # opens the HAM gate; group-0 DMA wait stays under the idle window
NFILL = 2          # keep-warm dummy matmuls per group

PROFILE = {"trace": False, "tmpdir": None}
LAST_RESULT = None

_BUILD_CACHE = {}


def _plan(n_nodes):
    per = -(-n_nodes // NCORES)
    nsc = -(-per // 512)
    groups = []
    left = nsc
    while left > SCG + 3:
        groups.append(SCG)
        left -= SCG
    while left > 0:
        s = 2 if left > 1 else left
        groups.append(s)
        left -= s
    return per, nsc, groups


def _build_inputs(z, batch_index, W1, b1, W2, b2):
    N = z.shape[0]
    per, nsc, groups = _plan(N)
    npad = nsc * 512
    nT = nsc * 4

    W1T = np.zeros((P, 384), dtype=ml_dtypes.bfloat16)
    w1t = np.ascontiguousarray(W1.T)
    for ch in range(3):
        k0, k1 = 128 * ch, min(128 * (ch + 1), IN_DIM)
        W1T[: k1 - k0, 128 * ch: 128 * ch + HID] = w1t[k0:k1]
    W2REP = np.tile(np.asarray(W2, np.float32).reshape(HID, 1),
                    (1, P)).astype(ml_dtypes.bfloat16)
    cf32 = np.zeros((P, 2), dtype=np.float32)
    cf32[:HID, 0] = np.asarray(b1, np.float32).reshape(-1)
    cf32[:, 1] = float(np.asarray(b2).reshape(-1)[0])
    # W1T2B: zth2 weights duplicated on partitions 64:128 so odd
    # superchunks' zth2 matmuls run in the 64:128 row-group (concurrent
    # with the even superchunk's matmul in rows 0:64)
    W1T2B = np.zeros((P, 128), dtype=ml_dtypes.bfloat16)
    W1T2B[64:128] = W1T[0:64, 256:384]
    cbf = np.concatenate([W1T, W2REP, W1T2B], axis=1)   # [P, 640]

    sc_base = np.concatenate([[0], np.cumsum(groups)])
    in_maps, meta = [], []
    for c in range(NCORES):
        lo = c * per
        n = min(per, N - lo)
        zp = np.zeros((npad, IN_DIM), dtype=np.float32)
        zp[:n] = z[lo: lo + n]

        zq = zp.reshape(nsc, 512, IN_DIM).transpose(0, 2, 1)     # s, f, j
        zth01 = zq[:, 0:256].reshape(nsc, 2, P, 512).transpose(
            0, 2, 1, 3).reshape(nsc, P, 1024)
        # zth2 row-packed: sc 2p -> partitions 0:64, sc 2p+1 -> 64:128
        npair = -(-nsc // 2)
        zth2 = np.zeros((P, npair * 512), dtype=ml_dtypes.bfloat16)
        z2f = zq[:, 256:320].astype(ml_dtypes.bfloat16)          # [nsc,64,512]
        zth2[0:64] = z2f[0::2].transpose(1, 0, 2).reshape(64, npair * 512)
        odd = z2f[1::2]
        zth2[64:128, : odd.shape[0] * 512] = \
            odd.transpose(1, 0, 2).reshape(64, odd.shape[0] * 512)
        znm = zp[:, :POOL].reshape(nsc, 4, P, POOL).transpose(
            0, 2, 1, 3).reshape(nsc, P, 1024)
        zc = np.concatenate(
            [zth01.astype(ml_dtypes.bfloat16),
             znm.astype(ml_dtypes.bfloat16)], axis=2)            # [nsc,P,2048]

        bi = np.full(npad, -1, dtype=np.int64)
        bi[:n] = batch_index[lo: lo + n]
        gbases = []
        loc = np.full(npad, -1, dtype=np.int64)
        for k, S in enumerate(groups):
            a = sc_base[k] * 512
            b = a + S * 512
            if a < n:
                gb = int(bi[a])
                lv = bi[a:b] - gb
                lv[bi[a:b] < 0] = -1
                assert lv.max() < GW, f"group spans {lv.max()+1} graphs > {GW}"
                loc[a:b] = lv
            else:
                gb = -1
            gbases.append(gb)

        # host-precomputed 0/1 one-hot, fp8 (exact): [nsc, P, 4*GW]
        oh01 = (loc[:, None] == np.arange(GW, dtype=np.int64)[None, :])
        oh01 = oh01.reshape(nsc, 4, P, GW).transpose(0, 2, 1, 3) \
            .reshape(nsc, P, 4 * GW).astype(ml_dtypes.float8_e4m3)

        in_maps.append({
            "zc": np.ascontiguousarray(zc),
            "z2": np.ascontiguousarray(zth2),
            "oh01": np.ascontiguousarray(oh01),
            "cbf": cbf,
            "cf32": cf32,
        })
        meta.append({"gbases": gbases, "n": n, "npads": npad - n})
    return in_maps, meta, (nsc, tuple(groups))


def _build_program(key):
    if key in _BUILD_CACHE:
        return _BUILD_CACHE[key]
    nsc, groups = key
    ngrp = len(groups)
    sc_base = np.concatenate([[0], np.cumsum(groups)])

    nc = bacc.Bacc("TRN2", target_bir_lowering=False, debug=False,
                   num_devices=NCORES)
    npair = -(-nsc // 2)
    zc_d = nc.dram_tensor("zc", [nsc, P, SCW], BF16, kind="ExternalInput").ap()
    z2_d = nc.dram_tensor("z2", [P, npair * 512], BF16,
                          kind="ExternalInput").ap()
    oh01_d = nc.dram_tensor("oh01", [nsc, P, 4 * GW], FP8,
                            kind="ExternalInput").ap()
    cbf_d = nc.dram_tensor("cbf", [P, 640], BF16, kind="ExternalInput").ap()
    cf32_d = nc.dram_tensor("cf32", [P, 2], F32, kind="ExternalInput").ap()
    slot_d = nc.dram_tensor("slot", [ngrp * P, POOL], BF16,
                            kind="ExternalOutput").ap()
    sacc_d = nc.dram_tensor("sacc", [P, ngrp], F32,
                            kind="ExternalOutput").ap()

    with tile.TileContext(nc) as tc:
        with tc.tile_pool(name="const", bufs=1) as cpool, \
             tc.tile_pool(name="zc", bufs=8) as zcpool, \
             tc.tile_pool(name="z2", bufs=8) as z2pool, \
             tc.tile_pool(name="o1", bufs=8) as o1pool, \
             tc.tile_pool(name="hs", bufs=8) as hpool, \
             tc.tile_pool(name="sT", bufs=2) as stpool, \
             tc.tile_pool(name="wc", bufs=3) as wcpool, \
             tc.tile_pool(name="oh", bufs=8) as ohpool, \
             tc.tile_pool(name="slot", bufs=2) as slotpool, \
             tc.tile_pool(name="ps_h", bufs=SCG, space="PSUM") as psh, \
             tc.tile_pool(name="ps_s", bufs=2, space="PSUM") as pss, \
             tc.tile_pool(name="ps_w", bufs=2, space="PSUM") as psw:

            # ---- constants ----
            cbf_sb = cpool.tile([P, 640], BF16)
            nc.gpsimd.dma_start(cbf_sb[:], cbf_d[:])
            cf32_sb = cpool.tile([P, 2], F32)
            nc.gpsimd.dma_start(cf32_sb[:], cf32_d[:])
            w1t_sb = cbf_sb[:, 0:384]
            w2rep_sb = cbf_sb[:, 384:512]
            w1t2b_sb = cbf_sb[:, 512:640]
            b1_sb = cf32_sb[0:HID, 0:1]
            b2s_sb = cf32_sb[:, 1:2]
            zero_sb = cpool.tile([P, 256], BF16)
            nc.vector.memset(zero_sb[:], 0.0)
            sacc_sb = cpool.tile([P, ngrp], F32)

            # ---- HAM warm-up: spans the group-0 DMA wait; [128, 1]
            # stationary keeps the LDWEIGHTS cost negligible ----
            warm_ps = pss.tile([P, 512], F32, tag="s", name="warm")
            for i in range(NWARM):
                nc.tensor.matmul(warm_ps[0:1, 0:256], zero_sb[:, 0:1],
                                 zero_sb[:, 0:256],
                                 start=(i == 0), stop=(i == NWARM - 1))

            def dma_group(k):
                S = groups[k]
                s0 = sc_base[k]
                zc = zcpool.tile([P, S * SCW], BF16, tag="zc", name="zc")
                if k < 2 and S > 1:
                    # ramp: land the first superchunks early so MLP(0)
                    # starts ~3.5us sooner
                    h = S // 2
                    nc.sync.dma_start(
                        zc[:, : h * SCW].rearrange("p (a b) -> p a b", a=h),
                        zc_d[s0: s0 + h].rearrange("a p b -> p a b"))
                    nc.sync.dma_start(
                        zc[:, h * SCW:].rearrange("p (a b) -> p a b", a=S - h),
                        zc_d[s0 + h: s0 + S].rearrange("a p b -> p a b"))
                else:
                    nc.sync.dma_start(
                        zc[:].rearrange("p (a b) -> p a b", a=S),
                        zc_d[s0: s0 + S].rearrange("a p b -> p a b"))
                sp = -(-S // 2)
                z2 = z2pool.tile([P, sp * 512], BF16, tag="z2", name="z2")
                nc.gpsimd.dma_start(
                    z2[:], z2_d[:, 512 * (s0 // 2): 512 * (s0 // 2 + sp)])
                o1 = o1pool.tile([P, S * 4 * GW], FP8, tag="o1", name="o1")
                nc.sync.dma_start(
                    o1[:].rearrange("p (a b) -> p a b", a=S),
                    oh01_d[s0: s0 + S].rearrange("a p b -> p a b"))
                return zc, z2, o1

            bufs = {}
            for j in range(min(4, ngrp)):
                bufs[j] = dma_group(j)

            prev1 = None   # group k-1: (zc, o1, S, hs, k)
            prev2 = None   # group k-2: (zc, S, ohs, k)
            for k in range(ngrp + 2):
                if k < ngrp:
                    if k + 4 < ngrp:
                        bufs[k + 4] = dma_group(k + 4)
                    zc, z2, o1 = bufs.pop(k)
                    S = groups[k]

                if 1 <= k <= 2:
                    fill_ps = pss.tile([P, 512], F32, tag="s", name="rampf")
                    for f in range(14):
                        nc.tensor.matmul(fill_ps[0:1, 0:256],
                                         zero_sb[:, 0:1], zero_sb[:, 0:256],
                                         start=(f == 0), stop=(f == 13))

                # ---- MLP for group k ----
                nprev2 = None
                if k < ngrp:
                    hpss = []
                    for i in range(S):
                        h_ps = psh.tile([P, 512], F32, tag="h", name="h")
                        nc.tensor.matmul(h_ps[:], w1t_sb[:, 0:128],
                                         zc[:, SCW * i: SCW * i + 512],
                                         start=True, stop=False)
                        hpss.append(h_ps)
                    for i in range(S):
                        nc.tensor.matmul(hpss[i][:], w1t_sb[:, 128:256],
                                         zc[:, SCW * i + 512: SCW * i + 1024],
                                         start=False, stop=False)
                    for p in range(-(-S // 2)):
                        nc.tensor.matmul(hpss[2 * p][:],
                                         w1t_sb[0:64, 256:384],
                                         z2[0:64, 512 * p: 512 * (p + 1)],
                                         start=False, stop=True)
                        if 2 * p + 1 < S:
                            nc.tensor.matmul(hpss[2 * p + 1][:],
                                             w1t2b_sb[64:128, :],
                                             z2[64:128, 512 * p: 512 * (p + 1)],
                                             start=False, stop=True)
                    hs = []
                    for i in range(S):
                        h_sb = hpool.tile([P, 512], BF16, tag="hs", name="hs")
                        with tc.high_priority(offset=64):
                            nc.scalar.activation(h_sb[:], hpss[i][:], AF.Relu,
                                                 bias=b1_sb)
                        hs.append(h_sb)

                # ---- scores + one-hots for group k-1 ----
                if prev1 is not None:
                    pzc, po1, pS, phs, pk = prev1
                    sT = stpool.tile([P, 512 * pS], F32, tag="sT", name="sT")
                    for i in range(pS):
                        s128 = pss.tile([P, 512], F32, tag="s", name="s128")
                        nc.tensor.matmul(s128[:], w2rep_sb[:], phs[i][:],
                                         start=True, stop=True)
                        with tc.high_priority(offset=64):
                            nc.vector.transpose(
                                sT[:, 512 * i: 512 * (i + 1)], s128[:])
                    s_cols = wcpool.tile([P, 4 * pS], F32, tag="sc",
                                         name="sc")
                    for a in range(4):
                        with tc.high_priority(offset=64):
                            nc.vector.tensor_copy(
                                s_cols[32 * a: 32 * a + 32, :],
                                sT[32 * a: 32 * a + 32, 32 * a:: 128])
                    w_cols = wcpool.tile([P, 4 * pS], BF16, tag="wc",
                                         name="wc")
                    with tc.high_priority(offset=64):
                        nc.scalar.activation(w_cols[:], s_cols[:], AF.Exp,
                                             bias=b2s_sb,
                                             accum_out=sacc_sb[:, pk: pk + 1])
                    ohs = []
                    for i in range(pS):
                        oh = ohpool.tile([P, 4 * GW], BF16, tag="oh",
                                         name="oh")
                        wb = w_cols[:, 4 * i: 4 * i + 4].unsqueeze(2) \
                            .to_broadcast([P, 4, GW])
                        with tc.high_priority(offset=64):
                            nc.gpsimd.tensor_tensor(
                                oh[:].rearrange("p (t j) -> p t j", t=4),
                                po1[:, 4 * GW * i: 4 * GW * (i + 1)]
                                .rearrange("p (t j) -> p t j", t=4),
                                wb, ALU.mult)
                        ohs.append(oh)
                    nprev2 = (pzc, pS, ohs, pk)

                # ---- segment matmuls for group k-2 ----
                if prev2 is not None:
                    szc, sS, sohs, sk = prev2
                    wsum = psw.tile([P, POOL], F32, tag="ws", name="wsum")
                    nfill = 16 if sk < 2 else (10 if sk < 4 else NFILL)
                    for f in range(nfill):
                        nc.tensor.matmul(wsum[0:1, :],
                                         zero_sb[:, 0:1], zero_sb[:, 0:256],
                                         start=(f == 0), stop=(f == nfill - 1))
                    # tiles run 2-at-a-time: even tiles -> array col-group
                    # 0 (psum partitions 0:64), odd tiles -> col-group 1
                    # (partitions 64:128); concurrent on the PE array.
                    ntile = 4 * sS
                    for i in range(sS):
                        for t in range(4):
                            lt = 4 * i + t
                            zoff = SCW * i + 1024 + POOL * t
                            base = GW * (lt % 2)
                            nc.tensor.matmul(
                                wsum[base: base + GW, :],
                                sohs[i][:, GW * t: GW * (t + 1)],
                                szc[:, zoff: zoff + POOL],
                                start=(lt < 2), stop=(lt >= ntile - 2))
                    # both 64-row halves ship out (bf16); host adds them
                    # (lane-aligned engines cannot fold partitions 64:128
                    # onto 0:64 on-device)
                    slot_sb = slotpool.tile([P, POOL], BF16, tag="slot",
                                            name="slot")
                    nc.scalar.activation(slot_sb[:], wsum[:], AF.Copy)
                    nc.sync.dma_start(slot_d[P * sk: P * (sk + 1), :],
                                      slot_sb[:])

                prev2 = nprev2
                prev1 = (zc, o1, S, hs, k) if k < ngrp else None

            nc.sync.dma_start(sacc_d[:], sacc_sb[:])

    nc.compile()
    _BUILD_CACHE[key] = nc
    return nc


def kernel(z, batch_index, W1, b1, W2, b2, num_graphs):
    global LAST_RESULT
    z = np.asarray(z, dtype=np.float32)
    batch_index = np.asarray(batch_index)
    G = int(num_graphs)

    in_maps, meta, key = _build_inputs(
        z, batch_index, np.asarray(W1), np.asarray(b1),
        np.asarray(W2), np.asarray(b2))
    nc = _build_program(key)

    res = run_bass_kernel_spmd(
        nc, in_maps, list(range(NCORES)),
        trace=PROFILE["trace"],
        **({"tmpdir": PROFILE["tmpdir"]} if PROFILE["tmpdir"] else {}))
    LAST_RESULT = res

    nsc, groups = key
    sums = np.zeros((G, POOL), dtype=np.float64)
    S = 0.0
    w_pad = float(np.exp(
        np.asarray(W2, np.float64) @ np.maximum(
            np.asarray(b1, np.float64), 0.0)
        + np.asarray(b2, np.float64))[0])
    for c in range(NCORES):
        slot = np.asarray(res.results[c]["slot"], np.float64)
        S += float(np.asarray(res.results[c]["sacc"], np.float64).sum())
        S -= meta[c]["npads"] * w_pad
        for k in range(len(groups)):
            gb = meta[c]["gbases"][k]
            if gb < 0:
                continue
            L = min(GW, G - gb)
            sums[gb: gb + L] += (slot[P * k: P * k + L]
                                 + slot[P * k + GW: P * k + GW + L])
    counts = np.maximum(np.bincount(batch_index, minlength=G), 1)
    out = sums / (S * counts[:, None])
    return out.astype(np.float32)
